# revision 12
# baseline (speedup 1.0000x reference)
"""Trainium2 Bass kernel for nn_CustomLayer_22428319220577.

Math (reference):
    G    = Gmin + (W - Wmin) * a,  a = (Gmax-Gmin)/(Wmax-Wmin)
    G_q  = round((W-Wmin)/(Wmax-Wmin)*15) * (Gmax-Gmin)/15 + Gmin
    Geff = 1/(1/G_q + Rp*((M-i)+(j+1)))
    C    = x @ Geff ;  I = x @ G_q
    coeff= (rowrange I)/(rowrange C + EPS)
    C2   = (C - rowmean C)*coeff + rowmean I
    out  = (C2 - rowsum(x)*b)/a + bias,  b = Gmin - a*Wmin

Reformulated (removes the /a cancellation amplification):
    P = (G_q  - rowmean_j G_q )/a        [1024,1024]
    Q = (Geff - rowmean_j Geff)/a        [1024,1024]
    m = (rowmean_j G_q - b)/a            [1024]
    A = x@P ; B = x@Q ; d = x@m
    coeff = rowrange(A) / (rowrange(B) + EPS/a)
    out   = coeff*B + d + bias

Sharding: data-parallel over batch. 8 cores, each takes 1024 rows of x,
replicates weight/bias, no collectives (row stats are per-sample).
"""
import os
import sys

sys.path.insert(0, "/opt/trn_rl_repo")

from contextlib import ExitStack

import numpy as np

import concourse.bass as bass
import concourse.tile as tile
from concourse import bacc, mybir
from concourse import bass_isa
from concourse.bass_utils import run_bass_kernel_spmd
from concourse.masks import make_identity

# problem constants (hardcoded per contract)
B_FULL, K, N = 8192, 1024, 1024
N_CORES = 8
B_SH = B_FULL // N_CORES          # 1024 rows per core
MT = B_SH // 128                  # 8 batch tiles per core
KB = K // 128                     # 8 k blocks

R_HRS, R_LRS, RP, BITS, EPS = 40000.0, 1000.0, 2.0, 4, 1e-8
GMIN, GMAX = 1.0 / R_HRS, 1.0 / R_LRS
LEVELS = float(2**BITS - 1)
C2_IMM = np.float32((GMAX - GMIN) / LEVELS)     # G_q = r*C2 + GMIN
INV_GSPAN = np.float64(1.0) / (GMAX - GMIN)

FP32 = mybir.dt.float32
F32R = mybir.dt.float32r
I32 = mybir.dt.int32

# matmul operand dtype: FP32 = exact (4 cyc/row), F32R = ~11-bit mantissa (1 cyc/row)
MM_DT = FP32 if os.environ.get("KMM", "f32r") == "f32" else F32R


def _build():
    nc = bacc.Bacc("TRN2", target_bir_lowering=False, debug=False,
                   num_devices=N_CORES)

    xs = nc.dram_tensor("xs", [B_SH, K], FP32, kind="ExternalInput").ap()
    w = nc.dram_tensor("w", [K, N], FP32, kind="ExternalInput").ap()
    bias_d = nc.dram_tensor("bias", [N], FP32, kind="ExternalInput").ap()
    offs_d = nc.dram_tensor("offs", [128, KB], FP32, kind="ExternalInput").ap()
    out_d = nc.dram_tensor("out", [B_SH, N], FP32, kind="ExternalOutput").ap()

    with tile.TileContext(nc) as tc, ExitStack() as ctx:
        consts = ctx.enter_context(tc.tile_pool(name="consts", bufs=1))
        wkeep = ctx.enter_context(tc.tile_pool(name="wkeep", bufs=1))
        wtiles = ctx.enter_context(tc.tile_pool(name="wtiles", bufs=2))
        stats = ctx.enter_context(tc.tile_pool(name="stats", bufs=1))
        xin = ctx.enter_context(tc.tile_pool(name="xin", bufs=3))
        xtsb = ctx.enter_context(tc.tile_pool(name="xtsb", bufs=3))
        bsb = ctx.enter_context(tc.tile_pool(name="bsb", bufs=3))
        outp = ctx.enter_context(tc.tile_pool(name="outp", bufs=3))
        mtst = ctx.enter_context(tc.tile_pool(name="mtst", bufs=4))
        ps_tr = ctx.enter_context(tc.tile_pool(name="ps_tr", bufs=2, space="PSUM"))
        ps_a = ctx.enter_context(tc.tile_pool(name="ps_a", bufs=1, space="PSUM"))
        ps_b = ctx.enter_context(tc.tile_pool(name="ps_b", bufs=1, space="PSUM"))
        ps_d = ctx.enter_context(tc.tile_pool(name="ps_d", bufs=1, space="PSUM"))

        # ---------- constants ----------
        ident = consts.tile([128, 128], FP32)
        make_identity(nc, ident[:])

        biasb = consts.tile([128, N], FP32)
        nc.sync.dma_start(
            out=biasb[:],
            in_=bass.AP(tensor=bias_d.tensor, offset=bias_d.offset,
                        ap=[[0, 128]] + bias_d.ap),
        )

        offs = consts.tile([128, KB], FP32)
        nc.sync.dma_start(out=offs[:], in_=offs_d)

        # Rpj[p, j] = RP*(j+1)  (same for all partitions)
        rpj_i = consts.tile([128, N], I32)
        nc.gpsimd.iota(rpj_i[:], pattern=[[1, N]], base=0, channel_multiplier=0)
        rpj = consts.tile([128, N], FP32)
        nc.vector.tensor_scalar(out=rpj[:], in0=rpj_i[:], scalar1=RP, scalar2=RP,
                                op0=mybir.AluOpType.mult, op1=mybir.AluOpType.add)

        # ---------- W load + global min/max ----------
        wkbs = []
        wmax8 = stats.tile([128, KB], FP32)
        wmin8 = stats.tile([128, KB], FP32)
        for kb in range(KB):
            wkb = wkeep.tile([128, N], FP32, tag=f"wkb{kb}")
            nc.sync.dma_start(out=wkb[:], in_=w[kb * 128:(kb + 1) * 128, :])
            wkbs.append(wkb)
            nc.vector.tensor_reduce(out=wmax8[:, kb:kb + 1], in_=wkb[:],
                                    axis=mybir.AxisListType.X,
                                    op=mybir.AluOpType.max)
            nc.vector.tensor_reduce(out=wmin8[:, kb:kb + 1], in_=wkb[:],
                                    axis=mybir.AxisListType.X,
                                    op=mybir.AluOpType.min)

        wmaxp = stats.tile([128, 1], FP32)
        nc.vector.tensor_reduce(out=wmaxp[:], in_=wmax8[:],
                                axis=mybir.AxisListType.X, op=mybir.AluOpType.max)
        wminp = stats.tile([128, 1], FP32)
        nc.vector.tensor_reduce(out=wminp[:], in_=wmin8[:],
                                axis=mybir.AxisListType.X, op=mybir.AluOpType.min)
        wmax_t = stats.tile([128, 1], FP32)
        nc.gpsimd.partition_all_reduce(wmax_t[:], wmaxp[:], channels=128,
                                       reduce_op=bass_isa.ReduceOp.max)
        wminn = stats.tile([128, 1], FP32)
        nc.vector.tensor_scalar_mul(wminn[:], wminp[:], -1.0)
        wminn_t = stats.tile([128, 1], FP32)
        nc.gpsimd.partition_all_reduce(wminn_t[:], wminn[:], channels=128,
                                       reduce_op=bass_isa.ReduceOp.max)
        wmin_t = stats.tile([128, 1], FP32)
        nc.vector.tensor_scalar_mul(wmin_t[:], wminn_t[:], -1.0)

        # scalars ([128,1] broadcast tiles)
        span = stats.tile([128, 1], FP32)
        nc.vector.tensor_tensor(out=span[:], in0=wmax_t[:], in1=wmin_t[:],
                                op=mybir.AluOpType.subtract)
        rspan = stats.tile([128, 1], FP32)
        nc.vector.reciprocal(rspan[:], span[:])
        c1 = stats.tile([128, 1], FP32)
        nc.vector.tensor_scalar_mul(c1[:], rspan[:], LEVELS)
        a_t = stats.tile([128, 1], FP32)
        nc.vector.tensor_scalar_mul(a_t[:], rspan[:], float(GMAX - GMIN))
        inva_t = stats.tile([128, 1], FP32)
        nc.vector.tensor_scalar_mul(inva_t[:], span[:], float(INV_GSPAN))
        b_t = stats.tile([128, 1], FP32)
        nc.vector.tensor_tensor(out=b_t[:], in0=a_t[:], in1=wmin_t[:],
                                op=mybir.AluOpType.mult)
        nc.vector.tensor_scalar(out=b_t[:], in0=b_t[:], scalar1=-1.0, scalar2=GMIN,
                                op0=mybir.AluOpType.mult, op1=mybir.AluOpType.add)
        eps_t = stats.tile([128, 1], FP32)
        nc.vector.tensor_scalar_mul(eps_t[:], inva_t[:], EPS)

        # ---------- per-k-block precompute: Z = [P | Q], m ----------
        zsb = consts.tile([128, KB, 2 * N], MM_DT)
        # d-matvec rhs: N=1 fp32r matmuls fail the ISA check, so pad to N=2
        m8 = consts.tile([128, KB, 2], MM_DT)
        nc.vector.memset(m8[:].bitcast(FP32), 0.0)
        for kb in range(KB):
            wkb = wkbs[kb]
            t15 = wtiles.tile([128, N], I32, tag="t15")
            nc.vector.tensor_scalar(out=t15[:], in0=wkb[:], scalar1=wmin_t[:],
                                    scalar2=c1[:], op0=mybir.AluOpType.subtract,
                                    op1=mybir.AluOpType.mult)
            gq = wtiles.tile([128, N], FP32, tag="gq")
            nc.vector.tensor_scalar(out=gq[:], in0=t15[:], scalar1=float(C2_IMM),
                                    scalar2=GMIN, op0=mybir.AluOpType.mult,
                                    op1=mybir.AluOpType.add)
            # u = rowmean(gq)
            u_r = mtst.tile([128, 1], FP32, tag="u_r")
            nc.vector.tensor_reduce(out=u_r[:], in_=gq[:],
                                    axis=mybir.AxisListType.X,
                                    op=mybir.AluOpType.add)
            u_t = mtst.tile([128, 1], FP32, tag="u_t")
            nc.vector.tensor_scalar_mul(u_t[:], u_r[:], 1.0 / N)
            # P = (gq - u)*inva  -> Z[:, kb, 0:N]
            nc.vector.tensor_scalar(out=zsb[:, kb, 0:N], in0=gq[:], scalar1=u_t[:],
                                    scalar2=inva_t[:],
                                    op0=mybir.AluOpType.subtract,
                                    op1=mybir.AluOpType.mult)
            # inv = 1/gq (exact-ish), den = inv + RP*(j+1) + offs[kb], geff = 1/den
            inv = wtiles.tile([128, N], FP32, tag="inv")
            scr = wtiles.tile([128, N], FP32, tag="scr")
            nc.vector.reciprocal_approx_accurate(inv[:], gq[:], scr[:])
            den = wtiles.tile([128, N], FP32, tag="den")
            nc.vector.affine_then_add(den[:], inv[:], rpj[:], 1.0,
                                      offs[:, kb:kb + 1])
            geff = wtiles.tile([128, N], FP32, tag="geff")
            nc.vector.reciprocal_approx_accurate(geff[:], den[:], scr[:])
            v_r = mtst.tile([128, 1], FP32, tag="v_r")
            nc.vector.tensor_reduce(out=v_r[:], in_=geff[:],
                                    axis=mybir.AxisListType.X,
                                    op=mybir.AluOpType.add)
            v_t = mtst.tile([128, 1], FP32, tag="v_t")
            nc.vector.tensor_scalar_mul(v_t[:], v_r[:], 1.0 / N)
            # Q = (geff - v)*inva -> Z[:, kb, N:2N]
            nc.vector.tensor_scalar(out=zsb[:, kb, N:2 * N], in0=geff[:],
                                    scalar1=v_t[:], scalar2=inva_t[:],
                                    op0=mybir.AluOpType.subtract,
                                    op1=mybir.AluOpType.mult)
            # m = (u - b)*inva
            mtmp = mtst.tile([128, 1], FP32, tag="mtmp")
            nc.vector.tensor_tensor(out=mtmp[:], in0=u_t[:], in1=b_t[:],
                                    op=mybir.AluOpType.subtract)
            nc.vector.tensor_tensor(out=m8[:, kb, 0:1], in0=mtmp[:],
                                    in1=inva_t[:], op=mybir.AluOpType.mult)

        # ---------- main loop over batch tiles ----------
        for mt in range(MT):
            xnat = xin.tile([128, K], FP32)
            nc.sync.dma_start(out=xnat[:], in_=xs[mt * 128:(mt + 1) * 128, :])

            xt = xtsb.tile([128, K], MM_DT)
            for half in range(2):
                ptr = ps_tr.tile([128, 512], FP32)
                for q in range(4):
                    c = half * 4 + q
                    nc.tensor.transpose(ptr[:, q * 128:(q + 1) * 128],
                                        xnat[:, c * 128:(c + 1) * 128], ident[:])
                nc.scalar.copy(xt[:, half * 512:(half + 1) * 512], ptr[:])

            pa = ps_a.tile([128, 2, 512], FP32)
            pb = ps_b.tile([128, 2, 512], FP32)
            pd = ps_d.tile([128, 2], FP32)
            for kb in range(KB):
                lhsT = xt[:, kb * 128:(kb + 1) * 128]
                st, sp = kb == 0, kb == KB - 1
                nc.tensor.matmul(pa[:, 0, :], lhsT, zsb[:, kb, 0:512],
                                 start=st, stop=sp)
                nc.tensor.matmul(pa[:, 1, :], lhsT, zsb[:, kb, 512:1024],
                                 start=st, stop=sp)
                nc.tensor.matmul(pb[:, 0, :], lhsT, zsb[:, kb, 1024:1536],
                                 start=st, stop=sp)
                nc.tensor.matmul(pb[:, 1, :], lhsT, zsb[:, kb, 1536:2048],
                                 start=st, stop=sp)
                nc.tensor.matmul(pd[:], lhsT, m8[:, kb, :],
                                 start=st, stop=sp)

            # ranges of A directly from PSUM
            amax = mtst.tile([128, 1], FP32, tag="amax")
            nc.vector.tensor_reduce(out=amax[:], in_=pa[:], axis=mybir.AxisListType.XY,
                                    op=mybir.AluOpType.max)
            amin = mtst.tile([128, 1], FP32, tag="amin")
            nc.vector.tensor_reduce(out=amin[:], in_=pa[:], axis=mybir.AxisListType.XY,
                                    op=mybir.AluOpType.min)
            # copy B to SBUF (frees PSUM), d too
            bs = bsb.tile([128, N], FP32)
            nc.scalar.copy(bs[:, 0:512], pb[:, 0, :])
            nc.scalar.copy(bs[:, 512:1024], pb[:, 1, :])
            d_t = mtst.tile([128, 1], FP32, tag="d_t")
            nc.scalar.copy(d_t[:], pd[:, 0:1])

            bmax = mtst.tile([128, 1], FP32, tag="bmax")
            nc.vector.tensor_reduce(out=bmax[:], in_=bs[:], axis=mybir.AxisListType.X,
                                    op=mybir.AluOpType.max)
            bmin = mtst.tile([128, 1], FP32, tag="bmin")
            nc.vector.tensor_reduce(out=bmin[:], in_=bs[:], axis=mybir.AxisListType.X,
                                    op=mybir.AluOpType.min)

            ra = mtst.tile([128, 1], FP32, tag="ra")
            nc.vector.tensor_tensor(out=ra[:], in0=amax[:], in1=amin[:],
                                    op=mybir.AluOpType.subtract)
            rbe = mtst.tile([128, 1], FP32, tag="rbe")
            nc.vector.tensor_scalar(out=rbe[:], in0=bmax[:], scalar1=bmin[:],
                                    scalar2=eps_t[:], op0=mybir.AluOpType.subtract,
                                    op1=mybir.AluOpType.add)
            rc = mtst.tile([128, 1], FP32, tag="rc")
            nc.vector.reciprocal(rc[:], rbe[:])
            coeff = mtst.tile([128, 1], FP32, tag="coeff")
            nc.vector.tensor_tensor(out=coeff[:], in0=ra[:], in1=rc[:],
                                    op=mybir.AluOpType.mult)

            # out = (B*coeff + d) + bias
            osb = outp.tile([128, N], FP32)
            nc.vector.affine_then_add(osb[:], bs[:], biasb[:], coeff[:], d_t[:])
            nc.sync.dma_start(out=out_d[mt * 128:(mt + 1) * 128, :], in_=osb[:])

    nc.compile()
    return nc


_NC_CACHE = None


def _get_nc():
    global _NC_CACHE
    if _NC_CACHE is None:
        _NC_CACHE = _build()
    return _NC_CACHE


def _offs_np():
    p = np.arange(128, dtype=np.float64)[:, None]
    kb = np.arange(KB, dtype=np.float64)[None, :]
    return (RP * (K - (kb * 128 + p))).astype(np.float32)


def kernel(x, weight, bias):
    x = np.ascontiguousarray(x, np.float32)
    weight = np.ascontiguousarray(weight, np.float32)
    bias = np.ascontiguousarray(bias, np.float32)
    nc = _get_nc()
    offs = _offs_np()
    in_maps = [
        {"xs": x[c * B_SH:(c + 1) * B_SH], "w": weight, "bias": bias, "offs": offs}
        for c in range(N_CORES)
    ]
    res = run_bass_kernel_spmd(nc, in_maps, core_ids=list(range(N_CORES)))
    return np.concatenate([res.results[c]["out"] for c in range(N_CORES)], axis=0)
